# revision 87
# baseline (speedup 1.0000x reference)
"""Causal self-attention (GQA + RoPE) on 8 Trainium2 NeuronCores.

Sharding: core c = (b, g) with b = c // 4 (batch), g = c % 4 (group of 4
consecutive Q heads; KV head g // 2). Each core computes the attention
output for its 4 heads and a partial out-projection through the matching
256-column slice of Wo. Host sums the 4 partials per batch and adds bo.

Per-core kernel, tuned for PE-stream continuity (all matmul operands
bf16, fp32 PSUM accumulation):
  - kv projection loops the contraction chunk outermost so matmuls start
    as soon as the first x chunk lands from HBM (first chunk split into
    512-col DMA pieces so the very first matmul starts even earlier)
  - QK uses zero-padded 128-partition stationaries (kz0/kz1): the PE
    streams 1 column/cycle only with 128 active partitions; a 64-row
    stationary would halve throughput (measured 427ns vs 216ns per
    512-col matmul)
  - attention is emitted as software-pipelined units (2 QK matmuls into
    a 1024-wide score PSUM -> one 1024-wide exp -> 2 AV matmuls) with a
    pending-work deque keeping consumers ~LAG units behind producers so
    the PE never waits on the Activation engine
  - causal diag handled by multiplying exp tiles with a 0/1 triangle on
    DVE (bf16 SBUF operands hit the 2x DVE mode), off the PE/Act path
  - softmax denominator via a ones-column appended to V (stationary
    sliced to 65 output partitions); per-head-pair normalization: den
    rows DMAd to a lane-parallel layout (recip commutes with the
    round-trip permutation), one reciprocal, one 2-row selector matmul
    broadcasting both heads' recips into a single PSUM, in-place muls
  - unnormalized y and den rows leave PSUM immediately so py banks
    recycle without waiting on the reciprocal chain
  - per-qb out-projection deferred and pumped as PE filler during the
    next qb's attention; in the final q-block the idle py bank
    double-buffers the out-proj PSUM and the Act engine shares the
    evacuation load; output written bf16 (host gathers in f32)
"""

import sys

for _p in ("/opt/trn_rl_repo", "/opt/pypackages"):
    if _p not in sys.path:
        sys.path.append(_p)

from collections import deque
from contextlib import ExitStack

import numpy as np

import concourse.bacc as bacc
import concourse.mybir as mybir
import concourse.tile as tile
from concourse.bass import ts
from concourse.bass_utils import run_bass_kernel_spmd

B, T, C = 2, 2048, 1024
HQ, HKV, HD = 16, 2, 64
F32 = mybir.dt.float32
F32R = mybir.dt.float32r
BF16 = mybir.dt.bfloat16
AF = mybir.ActivationFunctionType
NCC = C // 128  # 8 chunks of the contraction dim
SCALE = 1.0 / 64.0  # the reference's double 1/sqrt(64) scaling
LAG = 5  # pending-work units the PE consumer stream trails producers by


def _emit(nc, tc, ctx, d):
    sing = ctx.enter_context(tc.tile_pool(name="sing", bufs=1))

    xT_sb = sing.tile([128, NCC, T], BF16)
    wq_sb = sing.tile([128, NCC, 256], BF16)
    wkv_sb = sing.tile([128, NCC, 128], BF16)
    wo_sb = sing.tile([128, 2, C], BF16)
    bq_sb = sing.tile([128, 2], F32)
    bkv_sb = sing.tile([128, 1], F32)
    cos_sb = sing.tile([128, T], BF16)
    sin_sb = sing.tile([128, T], BF16)
    r2t_sb = sing.tile([128, 128], BF16)
    id_sb = sing.tile([128, 128], BF16)
    tri_sb = sing.tile([128, 128], BF16)
    bselp_sb = sing.tile([2, 128], F32R)
    qT_sb = sing.tile([128, 2, T], BF16)   # pair j: head 2j at parts 0:64, 2j+1 at 64:128
    kvT_sb = sing.tile([128, T], BF16)     # v at parts 0:64, k (pre-rope) at 64:128
    kz0_sb = sing.tile([128, T], BF16)     # roped k at 0:64, zeros at 64:128
    kz1_sb = sing.tile([128, T], BF16)     # zeros at 0:64, roped k at 64:128
    vA_sb = sing.tile([128, 16, 128], BF16)  # v[k-chunk, :64] + ones col + zero pad
    yT_sb = sing.tile([128, 2, T], BF16)   # normalized attention out, pair layout

    # input DMAs: small weights/tables first so the first projection
    # matmuls start as early as possible; x streams in behind them;
    # wo (needed only by the out-projection) goes last
    xr = d["xT"].ap().rearrange("(cc p) t -> p cc t", p=128)
    nc.sync.dma_start(out=wkv_sb[:], in_=d["wkv"].ap().rearrange("(cc p) m -> p cc m", p=128))
    nc.sync.dma_start(out=bkv_sb[:], in_=d["bkv"].ap())
    for piece in range(4):
        nc.sync.dma_start(out=xT_sb[:, 0, ts(piece, 512)], in_=xr[:, 0, ts(piece, 512)])
    for cc in range(1, NCC):
        nc.sync.dma_start(out=xT_sb[:, cc, :], in_=xr[:, cc, :])
    nc.sync.dma_start(out=wq_sb[:], in_=d["wq"].ap().rearrange("(cc p) m -> p cc m", p=128))
    nc.sync.dma_start(out=bq_sb[:], in_=d["bq"].ap())
    nc.sync.dma_start(out=r2t_sb[:], in_=d["r2t"].ap())
    nc.sync.dma_start(out=cos_sb[:], in_=d["cos2b"].ap())
    nc.sync.dma_start(out=sin_sb[:], in_=d["sin2b"].ap())
    nc.sync.dma_start(out=id_sb[:], in_=d["ident"].ap())
    nc.sync.dma_start(out=tri_sb[:], in_=d["tri"].ap())
    nc.sync.dma_start(out=bselp_sb[:], in_=d["bselp"].ap())
    nc.sync.dma_start(out=wo_sb[:], in_=d["wo"].ap().rearrange("(j p) c -> p j c", p=128))

    # ---- phase 1: projections, RoPE, v transpose ----
    with tc.tile_pool(name="pkv", bufs=1, space="PSUM") as pkv:
        # kv projection, contraction-chunk outermost: matmuls start on the
        # first x chunk instead of waiting for all of x
        pskv = [pkv.tile([128, 512], F32, tag=f"kv{ch}", name=f"pskv{ch}") for ch in range(4)]
        for cc in range(NCC):
            for ch in range(4):
                nc.tensor.matmul(
                    pskv[ch][:], wkv_sb[:, cc, :], xT_sb[:, cc, ts(ch, 512)],
                    start=(cc == 0), stop=(cc == NCC - 1),
                )
        for ch in range(4):
            nc.scalar.activation(
                out=kvT_sb[:, ts(ch, 512)], in_=pskv[ch][:],
                func=AF.Identity, bias=bkv_sb[:, 0:1], scale=1.0,
            )
    with tc.tile_pool(name="pq1", bufs=2, space="PSUM") as pqp, \
         tc.tile_pool(name="tmp1", bufs=2) as tmp1:
        # RoPE on k (lives at partitions 64:128); roped k assembled at
        # parts 0:64 of kz0 (zeros above), duplicated to parts 64:128 of
        # kz1 (zeros below) — 128-partition stationaries stream columns at
        # full rate, a 64-partition stationary would run at half rate
        for ch in range(4):
            pr = pqp.tile([128, 512], F32, tag="rot")
            nc.tensor.matmul(
                pr[0:64, :], r2t_sb[64:128, 64:128],
                kvT_sb[64:128, ts(ch, 512)], start=True, stop=True,
            )
            t1 = tmp1.tile([128, 512], F32, tag="t1")
            t2 = tmp1.tile([128, 512], F32, tag="t2")
            nc.vector.tensor_mul(t1[0:64, :], kvT_sb[64:128, ts(ch, 512)], cos_sb[64:128, ts(ch, 512)])
            nc.vector.tensor_mul(t2[0:64, :], pr[0:64, :], sin_sb[0:64, ts(ch, 512)])
            nc.vector.tensor_add(kz0_sb[0:64, ts(ch, 512)], t1[0:64, :], t2[0:64, :])
        nc.vector.memset(kz0_sb[64:128, :], 0.0)
        nc.vector.memset(kz1_sb[0:64, :], 0.0)
        nc.sync.dma_start(out=kz1_sb[64:128, :], in_=kz0_sb[0:64, :])
        # q projection + bias + RoPE (in pair layout)
        def q_proj_rope(j):
            for ch in range(4):
                ps = pqp.tile([128, 512], F32, tag="proj", bufs=3, name="ps")
                for cc in range(NCC):
                    nc.tensor.matmul(
                        ps[:], wq_sb[:, cc, ts(j, 128)], xT_sb[:, cc, ts(ch, 512)],
                        start=(cc == 0), stop=(cc == NCC - 1),
                    )
                qp = tmp1.tile([128, 512], BF16, tag="qp", name="qp")
                nc.scalar.activation(
                    out=qp[:], in_=ps[:],
                    func=AF.Identity, bias=bq_sb[:, j:j + 1], scale=1.0,
                )
                pr = pqp.tile([128, 512], F32, tag="rot", name="pr")
                nc.tensor.matmul(pr[:], r2t_sb[:], qp[:], start=True, stop=True)
                t1 = tmp1.tile([128, 512], F32, tag="t1", name="t1")
                t2 = tmp1.tile([128, 512], F32, tag="t2", name="t2")
                nc.vector.tensor_mul(t1[:], qp[:], cos_sb[:, ts(ch, 512)])
                nc.vector.tensor_mul(t2[:], pr[:], sin_sb[:, ts(ch, 512)])
                nc.vector.tensor_add(qT_sb[:, j, ts(ch, 512)], t1[:], t2[:])

        q_proj_rope(0)
        # v -> [Tk, 64] layout with ones column (for the denominator).
        # Emitted after q pair 0's rope: vA is first needed LAG units into
        # the attention stream, so these DVE copies must not delay the
        # rope adds that gate the very first QK matmul.
        for c16 in range(16):
            pv = pqp.tile([128, 64], BF16, tag="vt")
            nc.tensor.transpose(pv[:], kvT_sb[0:64, ts(c16, 128)], id_sb[0:64, 0:64])
            nc.vector.tensor_copy(vA_sb[:, c16, 0:64], pv[:])
        nc.vector.memset(vA_sb[:, :, 64:65], 1.0)
        nc.vector.memset(vA_sb[:, :, 65:128], 0.0)
        q_proj_rope(1)

    # ---- phase 2: attention, software-pipelined ----
    with tc.tile_pool(name="pps", bufs=2, space="PSUM") as pps, \
         tc.tile_pool(name="ppy", bufs=2, space="PSUM") as ppy, \
         tc.tile_pool(name="ppb", bufs=1, space="PSUM") as ppb, \
         tc.tile_pool(name="ppo", bufs=1, space="PSUM") as ppo, \
         tc.tile_pool(name="expp", bufs=6) as expp, \
         tc.tile_pool(name="ost", bufs=3) as ost, \
         tc.tile_pool(name="nrm", bufs=2) as nrm:
        pending = deque()

        def pump(n=1):
            for _ in range(n):
                if pending:
                    pending.popleft()()

        def drain():
            while len(pending) > LAG:
                pump()

        for qb in range(4):
            den_t = nrm.tile([1, 2, 2, 512], F32, tag="den")

            def mk_oproj_pair(tq, cf, qb=qb):
                # split out-projection: the j0 matmul only needs heads 0/1
                # (normalized by the first pair-recip), so it can fill the
                # PE while the second pair's reciprocal chain is in flight
                slot = {}

                def j0():
                    if qb == 3 and tq == 13:
                        # the score pool is idle in the tail — borrow a bank
                        # for two extra j0 prefills covering the final
                        # reciprocal round-trip
                        po = pps.tile([128, 1024], F32, tag="s", name="po")[:, 0:512]
                    else:
                        pool, tag = (ppy, "py") if qb == 3 and (tq + cf) % 2 else (ppo, "po")
                        po = pool.tile([128, 512], F32, tag=tag, name="po")
                    slot["po"] = po
                    nc.tensor.matmul(
                        po[:], yT_sb[:, 0, ts(tq, 128)], wo_sb[:, 0, ts(cf, 512)],
                        start=True, stop=False,
                    )

                def j1():
                    po = slot["po"]
                    nc.tensor.matmul(
                        po[:], yT_sb[:, 1, ts(tq, 128)], wo_sb[:, 1, ts(cf, 512)],
                        start=False, stop=True,
                    )
                    ob = ost.tile([128, 512], BF16, tag="ob", bufs=5)
                    if qb == 3 and (tq + cf) % 2:
                        nc.scalar.copy(out=ob[:], in_=po[:])
                    else:
                        nc.vector.tensor_copy(ob[:], po[:])
                    nc.sync.dma_start(out=d["out"].ap()[ts(tq, 128), ts(cf, 512)], in_=ob[:])

                return j0, j1

            tail_pairs = []
            for h in range(4):
                j, base = h // 2, (h % 2) * 64
                py = ppy.tile([128, 512], F32, tag="py")
                first_av = [True]

                def mk_av(py, kb, ecols, pycols, stop, first_av=first_av):
                    e_, (e0, e1) = ecols
                    p0, p1 = pycols

                    def go():
                        nc.tensor.matmul(
                            py[0:65, p0:p1], vA_sb[:, kb, 0:65], e_[:, e0:e1],
                            start=first_av[0], stop=stop,
                        )
                        first_av[0] = False
                    return go

                # full 128x512 blocks below the diagonal, two k-chunks per
                # unit sharing one 1024-wide score PSUM + exp
                kz = kz0_sb if h % 2 == 0 else kz1_sb
                for fk in range(2 * qb):
                    s_ = pps.tile([128, 1024], F32, tag="s")
                    e_ = expp.tile([128, 1024], BF16, tag="e")
                    for half in range(2):
                        kb = 2 * fk + half
                        nc.tensor.matmul(
                            s_[:, ts(half, 512)], kz[:, ts(kb, 128)],
                            qT_sb[:, j, ts(qb, 512)], start=True, stop=True,
                        )
                    nc.scalar.activation(out=e_[:], in_=s_[:], func=AF.Exp, scale=SCALE)
                    for half in range(2):
                        pending.append(mk_av(
                            py, 2 * fk + half, (e_, (512 * half, 512 * half + 512)),
                            (0, 512), stop=False))
                        drain()
                # diagonal band: k-chunk 4qb+r covers q in [(4qb+r)*128, (qb+1)*512);
                # its first 128 columns straddle the diagonal and get
                # tri-masked on the exp tile (GpSimd, SBUF side)
                for dpair in range(2):
                    s_ = pps.tile([128, 1024], F32, tag="s")
                    e_ = expp.tile([128, 1024], BF16, tag="e")
                    off = 0
                    offs = []
                    for idx in range(2):
                        r = 2 * dpair + idx
                        w = 512 - 128 * r
                        kb = 4 * qb + r
                        qoff = kb * 128
                        nc.tensor.matmul(
                            s_[:, off:off + w], kz[:, ts(kb, 128)],
                            qT_sb[:, j, qoff:qoff + w], start=True, stop=True,
                        )
                        offs.append(off)
                        off += w
                    nc.scalar.activation(out=e_[:, 0:off], in_=s_[:, 0:off], func=AF.Exp, scale=SCALE)
                    for idx in range(2):
                        o = offs[idx]
                        # GpSimd is idle and never queues: the mask gates the
                        # AV matmul, and on DVE it would sit behind norm/evac
                        # ops (measured 3us of PE head-of-line stall)
                        nc.gpsimd.tensor_mul(e_[:, o:o + 128], e_[:, o:o + 128], tri_sb[:])
                    for idx in range(2):
                        r = 2 * dpair + idx
                        w = 512 - 128 * r
                        kb = 4 * qb + r
                        pending.append(mk_av(
                            py, kb, (e_, (offs[idx], offs[idx] + w)),
                            (128 * r, 512), stop=(r == 3)))
                        drain()

                # early evacuation: unnormalized y + den row leave PSUM
                # immediately so the py bank recycles without waiting on the
                # reciprocal chain
                def mk_evac(py=py, j=j, h=h, base=base, qb=qb, den_t=den_t):
                    def go():
                        nc.vector.tensor_copy(
                            yT_sb[base:base + 64, j, ts(qb, 512)], py[0:64, :])
                        nc.vector.tensor_copy(den_t[0:1, h // 2, h % 2, :], py[64:65, :])
                    return go

                pending.append(mk_evac())
                drain()

                if qb == 3 and h == 3:
                    # cover the final pair's reciprocal round-trip with four
                    # j0 out-proj matmuls (heads 0/1 already normed)
                    for tq in (12, 13):
                        for cf in range(2):
                            pair = mk_oproj_pair(tq, cf)
                            tail_pairs.append(pair)
                            pending.append(pair[0])

                if h % 2 == 1:
                    # per-head-pair normalization: DMA the [1,1024] den strip
                    # into a lane-parallel layout (any linearization —
                    # elementwise recip commutes with the permutation and the
                    # inverse DMA restores order), reciprocal, DMA back with
                    # the two heads on partitions 0/1, broadcast both heads'
                    # recips into one PSUM via a 2-row selector matmul, then
                    # normalize in place
                    def mk_norm_pair(c=h // 2, den_t=den_t, qb=qb):
                        def go():
                            dtp = nrm.tile([128, 8], F32, tag="dtp")
                            nc.sync.dma_start(out=dtp[:], in_=den_t[0:1, c, :, :])
                            rtp = nrm.tile([128, 8], F32, tag="rtp")
                            nc.vector.reciprocal(rtp[:], dtp[:])
                            rdr = nrm.tile([2, 512], F32R, tag="rdr")
                            nc.sync.dma_start(out=rdr[0:2, :], in_=rtp[:].bitcast(F32R))
                            pb = ppb.tile([128, 512], F32, tag="pb", name="pb")
                            nc.tensor.matmul(
                                pb[:], bselp_sb[0:2, :], rdr[0:2, :],
                                start=True, stop=True,
                            )
                            for hh in (2 * c, 2 * c + 1):
                                jj, bb = hh // 2, (hh % 2) * 64
                                nc.vector.tensor_mul(
                                    yT_sb[bb:bb + 64, jj, ts(qb, 512)],
                                    yT_sb[bb:bb + 64, jj, ts(qb, 512)],
                                    pb[bb:bb + 64, :],
                                )
                        return go

                    pending.append(mk_norm_pair())
                    drain()

            # out projection for this q-block (all 4 heads now normalized),
            # deferred into the next q-block's PE stream as filler
            if qb == 3:
                # drain the two pre-issued j0 accumulations, then the rest
                # j0/j1 adjacent (po double-buffered across ppo/ppy)
                for pair in tail_pairs:
                    pending.append(pair[1])
                for tq in range(14, 16):
                    for cf in range(2):
                        pair = mk_oproj_pair(tq, cf)
                        pending.append(pair[0])
                        pending.append(pair[1])
            else:
                for tq in range(4 * qb, 4 * qb + 4):
                    for cf in range(2):
                        pair = mk_oproj_pair(tq, cf)
                        pending.append(pair[0])
                        pending.append(pair[1])
            drain()
        while pending:
            pump()


def build_program():
    nc = bacc.Bacc("TRN2", target_bir_lowering=False, debug=False, num_devices=8)
    d = {}
    BF_IN = {"xT", "wq", "wkv", "wo", "r2t", "ident", "tri"}
    for name, shape in [
        ("xT", [C, T]), ("wq", [C, 256]), ("wkv", [C, 128]),
        ("bq", [128, 2]), ("bkv", [128, 1]), ("wo", [256, C]),
        ("cos2b", [128, T]), ("sin2b", [128, T]), ("r2t", [128, 128]),
        ("ident", [128, 128]), ("tri", [128, 128]), ("bselp", [2, 128]),
    ]:
        dt = BF16 if name in (BF_IN | {"cos2b", "sin2b"}) else (F32R if name == "bselp" else F32)
        d[name] = nc.dram_tensor(name, shape, dt, kind="ExternalInput")
    d["out"] = nc.dram_tensor("out", [T, C], BF16, kind="ExternalOutput")
    with tile.TileContext(nc) as tc, ExitStack() as ctx:
        _emit(nc, tc, ctx, d)
    nc.compile()
    return nc


def host_prep(inputs):
    """Slice/transpose the full inputs into the 8 per-core input maps."""
    import ml_dtypes
    bf = lambda a: np.ascontiguousarray(a).astype(ml_dtypes.bfloat16)
    f = lambda a: np.ascontiguousarray(np.asarray(a, dtype=np.float32))
    x, rc = f(inputs["x"]), f(inputs["rope_cache"])
    Wq, bq = f(inputs["Wq"]), f(inputs["bq"])
    Wk, bk = f(inputs["Wk"]), f(inputs["bk"])
    Wv, bv = f(inputs["Wv"]), f(inputs["bv"])
    Wo = f(inputs["Wo"])

    cos2 = np.tile(np.repeat(rc[:, 1::2].T, 2, axis=0), (2, 1))  # [128, T]
    sin2 = np.tile(np.repeat(rc[:, 0::2].T, 2, axis=0), (2, 1))
    R2 = np.zeros((128, 128), np.float32)
    for i in range(64):
        R2[2 * i, 2 * i + 1] = -1.0
        R2[2 * i + 1, 2 * i] = 1.0
    r2t = np.ascontiguousarray(R2.T)
    ident = np.eye(128, dtype=np.float32)
    kk, qq = np.arange(128)[:, None], np.arange(128)[None, :]
    tri = (kk <= qq).astype(np.float32)
    bselp = np.zeros((2, 128), np.float32)
    bselp[0, 0:64] = 1.0
    bselp[1, 64:128] = 1.0

    in_maps = []
    for core in range(8):
        b, g = core // 4, core % 4
        kv = g // 2
        in_maps.append({
            "xT": bf(x[b].T),
            "wq": bf(Wq[256 * g:256 * (g + 1), :].T),
            "wkv": bf(np.concatenate(
                [Wv[64 * kv:64 * (kv + 1)].T, Wk[64 * kv:64 * (kv + 1)].T], axis=1)),
            "bq": np.ascontiguousarray(bq[256 * g:256 * (g + 1)].reshape(2, 128).T),
            "bkv": np.concatenate(
                [bv[64 * kv:64 * (kv + 1)], bk[64 * kv:64 * (kv + 1)]]).reshape(128, 1),
            "wo": bf(Wo[:, 256 * g:256 * (g + 1)].T),
            "cos2b": bf(cos2), "sin2b": bf(sin2), "r2t": bf(r2t),
            "ident": bf(ident), "tri": bf(tri), "bselp": bselp,
        })
    return in_maps


_PROGRAM = None


def _get_program():
    global _PROGRAM
    if _PROGRAM is None:
        _PROGRAM = build_program()
    return _PROGRAM


def _gather(results, bo):
    full = np.empty((B, T, C), np.float32)
    for b in range(B):
        acc = results[4 * b]["out"].astype(np.float32).copy()
        for g in range(1, 4):
            acc += results[4 * b + g]["out"]
        full[b] = acc + bo
    return full


def kernel(**inputs):
    nc = _get_program()
    in_maps = host_prep(inputs)
    res = run_bass_kernel_spmd(nc, in_maps, list(range(8)))
    return _gather(res.results, np.asarray(inputs["bo"], np.float32))


def kernel_traced(**inputs):
    """Like kernel() but with NTFF tracing; returns (output, BassKernelResults)."""
    nc = _get_program()
    in_maps = host_prep(inputs)
    res = run_bass_kernel_spmd(nc, in_maps, list(range(8)), trace=True)
    return _gather(res.results, np.asarray(inputs["bo"], np.float32)), res


# revision 88
# speedup vs baseline: 1.0119x; 1.0119x over previous
"""Causal self-attention (GQA + RoPE) on 8 Trainium2 NeuronCores.

Sharding: core c = (b, g) with b = c // 4 (batch), g = c % 4 (group of 4
consecutive Q heads; KV head g // 2). Each core computes the attention
output for its 4 heads and a partial out-projection through the matching
256-column slice of Wo. Host sums the 4 partials per batch and adds bo.

Per-core kernel, tuned for PE-stream continuity (all matmul operands
bf16, fp32 PSUM accumulation):
  - kv projection loops the contraction chunk outermost so matmuls start
    as soon as the first x chunk lands from HBM (first chunk split into
    512-col DMA pieces so the very first matmul starts even earlier)
  - QK uses zero-padded 128-partition stationaries (kz0/kz1): the PE
    streams 1 column/cycle only with 128 active partitions; a 64-row
    stationary would halve throughput (measured 427ns vs 216ns per
    512-col matmul)
  - attention is emitted as software-pipelined units (2 QK matmuls into
    a 1024-wide score PSUM -> one 1024-wide exp -> 2 AV matmuls) with a
    pending-work deque keeping consumers ~LAG units behind producers so
    the PE never waits on the Activation engine
  - causal diag handled by multiplying exp tiles with a 0/1 triangle on
    DVE (bf16 SBUF operands hit the 2x DVE mode), off the PE/Act path
  - softmax denominator via a ones-column appended to V (stationary
    sliced to 65 output partitions); per-head-pair normalization: den
    rows DMAd to a lane-parallel layout (recip commutes with the
    round-trip permutation), one reciprocal, one 2-row selector matmul
    broadcasting both heads' recips into a single PSUM, in-place muls
  - unnormalized y and den rows leave PSUM immediately so py banks
    recycle without waiting on the reciprocal chain
  - per-qb out-projection deferred and pumped as PE filler during the
    next qb's attention; in the final q-block the idle py bank
    double-buffers the out-proj PSUM and the Act engine shares the
    evacuation load; output written bf16 (host gathers in f32)
"""

import sys

for _p in ("/opt/trn_rl_repo", "/opt/pypackages"):
    if _p not in sys.path:
        sys.path.append(_p)

from collections import deque
from contextlib import ExitStack

import numpy as np

import concourse.bacc as bacc
import concourse.mybir as mybir
import concourse.tile as tile
from concourse.bass import ts
from concourse.bass_utils import run_bass_kernel_spmd

B, T, C = 2, 2048, 1024
HQ, HKV, HD = 16, 2, 64
F32 = mybir.dt.float32
F32R = mybir.dt.float32r
BF16 = mybir.dt.bfloat16
AF = mybir.ActivationFunctionType
NCC = C // 128  # 8 chunks of the contraction dim
SCALE = 1.0 / 64.0  # the reference's double 1/sqrt(64) scaling
LAG = 5  # pending-work units the PE consumer stream trails producers by


def _emit(nc, tc, ctx, d):
    sing = ctx.enter_context(tc.tile_pool(name="sing", bufs=1))

    xT_sb = sing.tile([128, NCC, T], BF16)
    wq_sb = sing.tile([128, NCC, 256], BF16)
    wkv_sb = sing.tile([128, NCC, 128], BF16)
    wo_sb = sing.tile([128, 2, C], BF16)
    bq_sb = sing.tile([128, 2], F32)
    bkv_sb = sing.tile([128, 1], F32)
    cos_sb = sing.tile([128, T], BF16)
    sin_sb = sing.tile([128, T], BF16)
    r2t_sb = sing.tile([128, 128], BF16)
    id_sb = sing.tile([128, 128], BF16)
    tri_sb = sing.tile([128, 128], BF16)
    bselp_sb = sing.tile([2, 128], F32R)
    qT_sb = sing.tile([128, 2, T], BF16)   # pair j: head 2j at parts 0:64, 2j+1 at 64:128
    kvT_sb = sing.tile([128, T], BF16)     # v at parts 0:64, k (pre-rope) at 64:128
    kz0_sb = sing.tile([128, T], BF16)     # roped k at 0:64, zeros at 64:128
    kz1_sb = sing.tile([128, T], BF16)     # zeros at 0:64, roped k at 64:128
    vA_sb = sing.tile([128, 16, 128], BF16)  # v[k-chunk, :64] + ones col + zero pad
    yT_sb = sing.tile([128, 2, T], BF16)   # normalized attention out, pair layout

    # input DMAs: small weights/tables first so the first projection
    # matmuls start as early as possible; x streams in behind them;
    # wo (needed only by the out-projection) goes last
    xr = d["xT"].ap().rearrange("(cc p) t -> p cc t", p=128)
    nc.sync.dma_start(out=wkv_sb[:], in_=d["wkv"].ap().rearrange("(cc p) m -> p cc m", p=128))
    nc.sync.dma_start(out=bkv_sb[:], in_=d["bkv"].ap())
    for piece in range(4):
        nc.sync.dma_start(out=xT_sb[:, 0, ts(piece, 512)], in_=xr[:, 0, ts(piece, 512)])
    for cc in range(1, NCC):
        nc.sync.dma_start(out=xT_sb[:, cc, :], in_=xr[:, cc, :])
    nc.sync.dma_start(out=wq_sb[:], in_=d["wq"].ap().rearrange("(cc p) m -> p cc m", p=128))
    nc.sync.dma_start(out=bq_sb[:], in_=d["bq"].ap())
    nc.sync.dma_start(out=r2t_sb[:], in_=d["r2t"].ap())
    nc.sync.dma_start(out=cos_sb[:], in_=d["cos2b"].ap())
    nc.sync.dma_start(out=sin_sb[:], in_=d["sin2b"].ap())
    nc.sync.dma_start(out=id_sb[:], in_=d["ident"].ap())
    nc.sync.dma_start(out=tri_sb[:], in_=d["tri"].ap())
    nc.sync.dma_start(out=bselp_sb[:], in_=d["bselp"].ap())
    nc.sync.dma_start(out=wo_sb[:], in_=d["wo"].ap().rearrange("(j p) c -> p j c", p=128))

    # ---- phase 1: projections, RoPE, v transpose ----
    with tc.tile_pool(name="pkv", bufs=1, space="PSUM") as pkv:
        # kv projection, contraction-chunk outermost: matmuls start on the
        # first x chunk instead of waiting for all of x
        pskv = [pkv.tile([128, 512], F32, tag=f"kv{ch}", name=f"pskv{ch}") for ch in range(4)]
        for cc in range(NCC):
            for ch in range(4):
                nc.tensor.matmul(
                    pskv[ch][:], wkv_sb[:, cc, :], xT_sb[:, cc, ts(ch, 512)],
                    start=(cc == 0), stop=(cc == NCC - 1),
                )
        for ch in range(4):
            nc.scalar.activation(
                out=kvT_sb[:, ts(ch, 512)], in_=pskv[ch][:],
                func=AF.Identity, bias=bkv_sb[:, 0:1], scale=1.0,
            )
    with tc.tile_pool(name="pq1", bufs=2, space="PSUM") as pqp, \
         tc.tile_pool(name="tmp1", bufs=2) as tmp1:
        # RoPE on k (lives at partitions 64:128); roped k assembled at
        # parts 0:64 of kz0 (zeros above), duplicated to parts 64:128 of
        # kz1 (zeros below) — 128-partition stationaries stream columns at
        # full rate, a 64-partition stationary would run at half rate
        for ch in range(4):
            pr = pqp.tile([128, 512], F32, tag="rot")
            nc.tensor.matmul(
                pr[0:64, :], r2t_sb[64:128, 64:128],
                kvT_sb[64:128, ts(ch, 512)], start=True, stop=True,
            )
            t1 = tmp1.tile([128, 512], F32, tag="t1")
            t2 = tmp1.tile([128, 512], F32, tag="t2")
            nc.vector.tensor_mul(t1[0:64, :], kvT_sb[64:128, ts(ch, 512)], cos_sb[64:128, ts(ch, 512)])
            nc.vector.tensor_mul(t2[0:64, :], pr[0:64, :], sin_sb[0:64, ts(ch, 512)])
            nc.vector.tensor_add(kz0_sb[0:64, ts(ch, 512)], t1[0:64, :], t2[0:64, :])
        nc.vector.memset(kz0_sb[64:128, :], 0.0)
        nc.vector.memset(kz1_sb[0:64, :], 0.0)
        nc.sync.dma_start(out=kz1_sb[64:128, :], in_=kz0_sb[0:64, :])
        # q projection + bias + RoPE (in pair layout)
        def q_proj_rope(j):
            for ch in range(4):
                ps = pqp.tile([128, 512], F32, tag="proj", bufs=3, name="ps")
                for cc in range(NCC):
                    nc.tensor.matmul(
                        ps[:], wq_sb[:, cc, ts(j, 128)], xT_sb[:, cc, ts(ch, 512)],
                        start=(cc == 0), stop=(cc == NCC - 1),
                    )
                qp = tmp1.tile([128, 512], BF16, tag="qp", name="qp")
                nc.scalar.activation(
                    out=qp[:], in_=ps[:],
                    func=AF.Identity, bias=bq_sb[:, j:j + 1], scale=1.0,
                )
                pr = pqp.tile([128, 512], F32, tag="rot", name="pr")
                nc.tensor.matmul(pr[:], r2t_sb[:], qp[:], start=True, stop=True)
                t1 = tmp1.tile([128, 512], F32, tag="t1", name="t1")
                t2 = tmp1.tile([128, 512], F32, tag="t2", name="t2")
                nc.vector.tensor_mul(t1[:], qp[:], cos_sb[:, ts(ch, 512)])
                nc.vector.tensor_mul(t2[:], pr[:], sin_sb[:, ts(ch, 512)])
                nc.vector.tensor_add(qT_sb[:, j, ts(ch, 512)], t1[:], t2[:])

        q_proj_rope(0)
        # v -> [Tk, 64] layout with ones column (for the denominator).
        # Emitted after q pair 0's rope: vA is first needed LAG units into
        # the attention stream, so these DVE copies must not delay the
        # rope adds that gate the very first QK matmul.
        for c16 in range(16):
            pv = pqp.tile([128, 64], BF16, tag="vt")
            nc.tensor.transpose(pv[:], kvT_sb[0:64, ts(c16, 128)], id_sb[0:64, 0:64])
            nc.vector.tensor_copy(vA_sb[:, c16, 0:64], pv[:])
        nc.vector.memset(vA_sb[:, :, 64:65], 1.0)
        nc.vector.memset(vA_sb[:, :, 65:128], 0.0)
        q_proj_rope(1)

    # ---- phase 2: attention, software-pipelined ----
    with tc.tile_pool(name="pps", bufs=2, space="PSUM") as pps, \
         tc.tile_pool(name="ppy", bufs=2, space="PSUM") as ppy, \
         tc.tile_pool(name="ppb", bufs=1, space="PSUM") as ppb, \
         tc.tile_pool(name="ppo", bufs=1, space="PSUM") as ppo, \
         tc.tile_pool(name="expp", bufs=6) as expp, \
         tc.tile_pool(name="ost", bufs=3) as ost, \
         tc.tile_pool(name="nrm", bufs=2) as nrm:
        pending = deque()

        def pump(n=1):
            for _ in range(n):
                if pending:
                    pending.popleft()()

        def drain():
            while len(pending) > LAG:
                pump()

        for qb in range(4):
            den_t = nrm.tile([1, 2, 2, 512], F32, tag="den")

            def mk_oproj_pair(tq, cf, qb=qb):
                # split out-projection: the j0 matmul only needs heads 0/1
                # (normalized by the first pair-recip), so it can fill the
                # PE while the second pair's reciprocal chain is in flight
                slot = {}

                def j0():
                    if qb == 3 and tq == 13:
                        # the score pool is idle in the tail — borrow a bank
                        # for two extra j0 prefills covering the final
                        # reciprocal round-trip
                        po = pps.tile([128, 1024], F32, tag="s", name="po")[:, 0:512]
                    else:
                        pool, tag = (ppy, "py") if qb == 3 and (tq + cf) % 2 else (ppo, "po")
                        po = pool.tile([128, 512], F32, tag=tag, name="po")
                    slot["po"] = po
                    nc.tensor.matmul(
                        po[:], yT_sb[:, 0, ts(tq, 128)], wo_sb[:, 0, ts(cf, 512)],
                        start=True, stop=False,
                    )

                def j1():
                    po = slot["po"]
                    nc.tensor.matmul(
                        po[:], yT_sb[:, 1, ts(tq, 128)], wo_sb[:, 1, ts(cf, 512)],
                        start=False, stop=True,
                    )
                    ob = ost.tile([128, 512], BF16, tag="ob", bufs=5)
                    if qb == 3 and (tq + cf) % 2:
                        nc.scalar.copy(out=ob[:], in_=po[:])
                    else:
                        nc.vector.tensor_copy(ob[:], po[:])
                    nc.sync.dma_start(out=d["out"].ap()[ts(tq, 128), ts(cf, 512)], in_=ob[:])

                return j0, j1

            tail_pairs = []
            for h in range(4):
                j, base = h // 2, (h % 2) * 64
                py = ppy.tile([128, 512], F32, tag="py")
                first_av = [True]

                def mk_av(py, kb, ecols, pycols, stop, first_av=first_av):
                    e_, (e0, e1) = ecols
                    p0, p1 = pycols

                    def go():
                        nc.tensor.matmul(
                            py[0:65, p0:p1], vA_sb[:, kb, 0:65], e_[:, e0:e1],
                            start=first_av[0], stop=stop,
                        )
                        first_av[0] = False
                    return go

                # full 128x512 blocks below the diagonal, two k-chunks per
                # unit sharing one 1024-wide score PSUM + exp
                kz = kz0_sb if h % 2 == 0 else kz1_sb
                for fk in range(2 * qb):
                    s_ = pps.tile([128, 1024], F32, tag="s")
                    e_ = expp.tile([128, 1024], BF16, tag="e")
                    for half in range(2):
                        kb = 2 * fk + half
                        nc.tensor.matmul(
                            s_[:, ts(half, 512)], kz[:, ts(kb, 128)],
                            qT_sb[:, j, ts(qb, 512)], start=True, stop=True,
                        )
                    nc.scalar.activation(out=e_[:], in_=s_[:], func=AF.Exp, scale=SCALE)
                    for half in range(2):
                        pending.append(mk_av(
                            py, 2 * fk + half, (e_, (512 * half, 512 * half + 512)),
                            (0, 512), stop=False))
                        drain()
                # diagonal band: k-chunk 4qb+r covers q in [(4qb+r)*128, (qb+1)*512);
                # its first 128 columns straddle the diagonal and get
                # tri-masked on the exp tile (GpSimd, SBUF side)
                for dpair in range(2):
                    s_ = pps.tile([128, 1024], F32, tag="s")
                    e_ = expp.tile([128, 1024], BF16, tag="e")
                    off = 0
                    offs = []
                    for idx in range(2):
                        r = 2 * dpair + idx
                        w = 512 - 128 * r
                        kb = 4 * qb + r
                        qoff = kb * 128
                        nc.tensor.matmul(
                            s_[:, off:off + w], kz[:, ts(kb, 128)],
                            qT_sb[:, j, qoff:qoff + w], start=True, stop=True,
                        )
                        offs.append(off)
                        off += w
                    nc.scalar.activation(out=e_[:, 0:off], in_=s_[:, 0:off], func=AF.Exp, scale=SCALE)
                    for idx in range(2):
                        o = offs[idx]
                        nc.vector.tensor_mul(e_[:, o:o + 128], e_[:, o:o + 128], tri_sb[:])
                    for idx in range(2):
                        r = 2 * dpair + idx
                        w = 512 - 128 * r
                        kb = 4 * qb + r
                        pending.append(mk_av(
                            py, kb, (e_, (offs[idx], offs[idx] + w)),
                            (128 * r, 512), stop=(r == 3)))
                        drain()

                # early evacuation: unnormalized y + den row leave PSUM
                # immediately so the py bank recycles without waiting on the
                # reciprocal chain
                def mk_evac(py=py, j=j, h=h, base=base, qb=qb, den_t=den_t):
                    def go():
                        nc.vector.tensor_copy(
                            yT_sb[base:base + 64, j, ts(qb, 512)], py[0:64, :])
                        nc.vector.tensor_copy(den_t[0:1, h // 2, h % 2, :], py[64:65, :])
                    return go

                pending.append(mk_evac())
                drain()

                if qb == 3 and h == 3:
                    # cover the final pair's reciprocal round-trip with four
                    # j0 out-proj matmuls (heads 0/1 already normed)
                    for tq in (12, 13):
                        for cf in range(2):
                            pair = mk_oproj_pair(tq, cf)
                            tail_pairs.append(pair)
                            pending.append(pair[0])

                if h % 2 == 1:
                    # per-head-pair normalization: DMA the [1,1024] den strip
                    # into a lane-parallel layout (any linearization —
                    # elementwise recip commutes with the permutation and the
                    # inverse DMA restores order), reciprocal, DMA back with
                    # the two heads on partitions 0/1, broadcast both heads'
                    # recips into one PSUM via a 2-row selector matmul, then
                    # normalize in place
                    def mk_norm_pair(c=h // 2, den_t=den_t, qb=qb):
                        def go():
                            dtp = nrm.tile([128, 8], F32, tag="dtp")
                            nc.sync.dma_start(out=dtp[:], in_=den_t[0:1, c, :, :])
                            rtp = nrm.tile([128, 8], F32, tag="rtp")
                            nc.vector.reciprocal(rtp[:], dtp[:])
                            rdr = nrm.tile([2, 512], F32R, tag="rdr")
                            nc.sync.dma_start(out=rdr[0:2, :], in_=rtp[:].bitcast(F32R))
                            pb = ppb.tile([128, 512], F32, tag="pb", name="pb")
                            nc.tensor.matmul(
                                pb[:], bselp_sb[0:2, :], rdr[0:2, :],
                                start=True, stop=True,
                            )
                            for hh in (2 * c, 2 * c + 1):
                                jj, bb = hh // 2, (hh % 2) * 64
                                nc.vector.tensor_mul(
                                    yT_sb[bb:bb + 64, jj, ts(qb, 512)],
                                    yT_sb[bb:bb + 64, jj, ts(qb, 512)],
                                    pb[bb:bb + 64, :],
                                )
                        return go

                    pending.append(mk_norm_pair())
                    drain()

            # out projection for this q-block (all 4 heads now normalized),
            # deferred into the next q-block's PE stream as filler
            if qb == 3:
                # drain the two pre-issued j0 accumulations, then the rest
                # j0/j1 adjacent (po double-buffered across ppo/ppy)
                for pair in tail_pairs:
                    pending.append(pair[1])
                for tq in range(14, 16):
                    for cf in range(2):
                        pair = mk_oproj_pair(tq, cf)
                        pending.append(pair[0])
                        pending.append(pair[1])
            else:
                for tq in range(4 * qb, 4 * qb + 4):
                    for cf in range(2):
                        pair = mk_oproj_pair(tq, cf)
                        pending.append(pair[0])
                        pending.append(pair[1])
            drain()
        while pending:
            pump()


def build_program():
    nc = bacc.Bacc("TRN2", target_bir_lowering=False, debug=False, num_devices=8)
    d = {}
    BF_IN = {"xT", "wq", "wkv", "wo", "r2t", "ident", "tri"}
    for name, shape in [
        ("xT", [C, T]), ("wq", [C, 256]), ("wkv", [C, 128]),
        ("bq", [128, 2]), ("bkv", [128, 1]), ("wo", [256, C]),
        ("cos2b", [128, T]), ("sin2b", [128, T]), ("r2t", [128, 128]),
        ("ident", [128, 128]), ("tri", [128, 128]), ("bselp", [2, 128]),
    ]:
        dt = BF16 if name in (BF_IN | {"cos2b", "sin2b"}) else (F32R if name == "bselp" else F32)
        d[name] = nc.dram_tensor(name, shape, dt, kind="ExternalInput")
    d["out"] = nc.dram_tensor("out", [T, C], BF16, kind="ExternalOutput")
    with tile.TileContext(nc) as tc, ExitStack() as ctx:
        _emit(nc, tc, ctx, d)
    nc.compile()
    return nc


def host_prep(inputs):
    """Slice/transpose the full inputs into the 8 per-core input maps."""
    import ml_dtypes
    bf = lambda a: np.ascontiguousarray(a).astype(ml_dtypes.bfloat16)
    f = lambda a: np.ascontiguousarray(np.asarray(a, dtype=np.float32))
    x, rc = f(inputs["x"]), f(inputs["rope_cache"])
    Wq, bq = f(inputs["Wq"]), f(inputs["bq"])
    Wk, bk = f(inputs["Wk"]), f(inputs["bk"])
    Wv, bv = f(inputs["Wv"]), f(inputs["bv"])
    Wo = f(inputs["Wo"])

    cos2 = np.tile(np.repeat(rc[:, 1::2].T, 2, axis=0), (2, 1))  # [128, T]
    sin2 = np.tile(np.repeat(rc[:, 0::2].T, 2, axis=0), (2, 1))
    R2 = np.zeros((128, 128), np.float32)
    for i in range(64):
        R2[2 * i, 2 * i + 1] = -1.0
        R2[2 * i + 1, 2 * i] = 1.0
    r2t = np.ascontiguousarray(R2.T)
    ident = np.eye(128, dtype=np.float32)
    kk, qq = np.arange(128)[:, None], np.arange(128)[None, :]
    tri = (kk <= qq).astype(np.float32)
    bselp = np.zeros((2, 128), np.float32)
    bselp[0, 0:64] = 1.0
    bselp[1, 64:128] = 1.0

    in_maps = []
    for core in range(8):
        b, g = core // 4, core % 4
        kv = g // 2
        in_maps.append({
            "xT": bf(x[b].T),
            "wq": bf(Wq[256 * g:256 * (g + 1), :].T),
            "wkv": bf(np.concatenate(
                [Wv[64 * kv:64 * (kv + 1)].T, Wk[64 * kv:64 * (kv + 1)].T], axis=1)),
            "bq": np.ascontiguousarray(bq[256 * g:256 * (g + 1)].reshape(2, 128).T),
            "bkv": np.concatenate(
                [bv[64 * kv:64 * (kv + 1)], bk[64 * kv:64 * (kv + 1)]]).reshape(128, 1),
            "wo": bf(Wo[:, 256 * g:256 * (g + 1)].T),
            "cos2b": bf(cos2), "sin2b": bf(sin2), "r2t": bf(r2t),
            "ident": bf(ident), "tri": bf(tri), "bselp": bselp,
        })
    return in_maps


_PROGRAM = None


def _get_program():
    global _PROGRAM
    if _PROGRAM is None:
        _PROGRAM = build_program()
    return _PROGRAM


def _gather(results, bo):
    full = np.empty((B, T, C), np.float32)
    for b in range(B):
        acc = results[4 * b]["out"].astype(np.float32).copy()
        for g in range(1, 4):
            acc += results[4 * b + g]["out"]
        full[b] = acc + bo
    return full


def kernel(**inputs):
    nc = _get_program()
    in_maps = host_prep(inputs)
    res = run_bass_kernel_spmd(nc, in_maps, list(range(8)))
    return _gather(res.results, np.asarray(inputs["bo"], np.float32))


def kernel_traced(**inputs):
    """Like kernel() but with NTFF tracing; returns (output, BassKernelResults)."""
    nc = _get_program()
    in_maps = host_prep(inputs)
    res = run_bass_kernel_spmd(nc, in_maps, list(range(8)), trace=True)
    return _gather(res.results, np.asarray(inputs["bo"], np.float32)), res


# revision 90
# speedup vs baseline: 1.0142x; 1.0023x over previous
"""Causal self-attention (GQA + RoPE) on 8 Trainium2 NeuronCores.

Sharding: core c = (b, g) with b = c // 4 (batch), g = c % 4 (group of 4
consecutive Q heads; KV head g // 2). Each core computes the attention
output for its 4 heads and a partial out-projection through the matching
256-column slice of Wo. Host sums the 4 partials per batch and adds bo.

Per-core kernel, tuned for PE-stream continuity (all matmul operands
bf16, fp32 PSUM accumulation):
  - kv projection loops the contraction chunk outermost so matmuls start
    as soon as the first x chunk lands from HBM (first chunk split into
    512-col DMA pieces so the very first matmul starts even earlier)
  - QK uses zero-padded 128-partition stationaries (kz0/kz1): the PE
    streams 1 column/cycle only with 128 active partitions; a 64-row
    stationary would halve throughput (measured 427ns vs 216ns per
    512-col matmul)
  - attention is emitted as software-pipelined units (2 QK matmuls into
    a 1024-wide score PSUM -> one 1024-wide exp -> 2 AV matmuls) with a
    pending-work deque keeping consumers ~LAG units behind producers so
    the PE never waits on the Activation engine
  - causal diag handled by multiplying exp tiles with a 0/1 triangle on
    DVE (bf16 SBUF operands hit the 2x DVE mode), off the PE/Act path
  - softmax denominator via a ones-column appended to V (stationary
    sliced to 65 output partitions); per-head-pair normalization: den
    rows DMAd to a lane-parallel layout (recip commutes with the
    round-trip permutation), one reciprocal, one 2-row selector matmul
    broadcasting both heads' recips into a single PSUM, in-place muls
  - unnormalized y and den rows leave PSUM immediately so py banks
    recycle without waiting on the reciprocal chain
  - per-qb out-projection deferred and pumped as PE filler during the
    next qb's attention; in the final q-block the idle py bank
    double-buffers the out-proj PSUM and the Act engine shares the
    evacuation load; output written bf16 (host gathers in f32)
"""

import sys

for _p in ("/opt/trn_rl_repo", "/opt/pypackages"):
    if _p not in sys.path:
        sys.path.append(_p)

from collections import deque
from contextlib import ExitStack

import numpy as np

import concourse.bacc as bacc
import concourse.mybir as mybir
import concourse.tile as tile
from concourse.bass import ts
from concourse.bass_utils import run_bass_kernel_spmd

B, T, C = 2, 2048, 1024
HQ, HKV, HD = 16, 2, 64
F32 = mybir.dt.float32
F32R = mybir.dt.float32r
BF16 = mybir.dt.bfloat16
AF = mybir.ActivationFunctionType
NCC = C // 128  # 8 chunks of the contraction dim
SCALE = 1.0 / 64.0  # the reference's double 1/sqrt(64) scaling
LAG = 5  # pending-work units the PE consumer stream trails producers by


def _emit(nc, tc, ctx, d):
    sing = ctx.enter_context(tc.tile_pool(name="sing", bufs=1))

    xT_sb = sing.tile([128, NCC, T], BF16)
    wq_sb = sing.tile([128, NCC, 256], BF16)
    wkv_sb = sing.tile([128, NCC, 128], BF16)
    wo_sb = sing.tile([128, 2, C], BF16)
    bq_sb = sing.tile([128, 2], F32)
    bkv_sb = sing.tile([128, 1], F32)
    cos_sb = sing.tile([128, T], BF16)
    sin_sb = sing.tile([128, T], BF16)
    r2t_sb = sing.tile([128, 128], BF16)
    id_sb = sing.tile([128, 128], BF16)
    tri_sb = sing.tile([128, 128], BF16)
    bselp_sb = sing.tile([2, 128], F32R)
    qT_sb = sing.tile([128, 2, T], BF16)   # pair j: head 2j at parts 0:64, 2j+1 at 64:128
    kvT_sb = sing.tile([128, T], BF16)     # v at parts 0:64, k (pre-rope) at 64:128
    kz0_sb = sing.tile([128, T], BF16)     # roped k at 0:64, zeros at 64:128
    kz1_sb = sing.tile([128, T], BF16)     # zeros at 0:64, roped k at 64:128
    vA_sb = sing.tile([128, 16, 128], BF16)  # v[k-chunk, :64] + ones col + zero pad
    yT_sb = sing.tile([128, 2, T], BF16)   # normalized attention out, pair layout

    # input DMAs: small weights/tables first so the first projection
    # matmuls start as early as possible; x streams in behind them;
    # wo (needed only by the out-projection) goes last
    xr = d["xT"].ap().rearrange("(cc p) t -> p cc t", p=128)
    nc.sync.dma_start(out=wkv_sb[:], in_=d["wkv"].ap().rearrange("(cc p) m -> p cc m", p=128))
    nc.sync.dma_start(out=bkv_sb[:], in_=d["bkv"].ap())
    for piece in range(4):
        nc.sync.dma_start(out=xT_sb[:, 0, ts(piece, 512)], in_=xr[:, 0, ts(piece, 512)])
    for cc in range(1, NCC):
        nc.sync.dma_start(out=xT_sb[:, cc, :], in_=xr[:, cc, :])
    nc.sync.dma_start(out=wq_sb[:], in_=d["wq"].ap().rearrange("(cc p) m -> p cc m", p=128))
    nc.sync.dma_start(out=bq_sb[:], in_=d["bq"].ap())
    nc.sync.dma_start(out=r2t_sb[:], in_=d["r2t"].ap())
    nc.sync.dma_start(out=cos_sb[:], in_=d["cos2b"].ap())
    nc.sync.dma_start(out=sin_sb[:], in_=d["sin2b"].ap())
    nc.sync.dma_start(out=id_sb[:], in_=d["ident"].ap())
    nc.sync.dma_start(out=tri_sb[:], in_=d["tri"].ap())
    nc.sync.dma_start(out=bselp_sb[:], in_=d["bselp"].ap())
    nc.sync.dma_start(out=wo_sb[:], in_=d["wo"].ap().rearrange("(j p) c -> p j c", p=128))

    # ---- phase 1: projections, RoPE, v transpose ----
    with tc.tile_pool(name="pkv", bufs=1, space="PSUM") as pkv:
        # kv projection, contraction-chunk outermost: matmuls start on the
        # first x chunk instead of waiting for all of x
        pskv = [pkv.tile([128, 512], F32, tag=f"kv{ch}", name=f"pskv{ch}") for ch in range(4)]
        for cc in range(NCC):
            for ch in range(4):
                nc.tensor.matmul(
                    pskv[ch][:], wkv_sb[:, cc, :], xT_sb[:, cc, ts(ch, 512)],
                    start=(cc == 0), stop=(cc == NCC - 1),
                )
        for ch in range(4):
            nc.scalar.activation(
                out=kvT_sb[:, ts(ch, 512)], in_=pskv[ch][:],
                func=AF.Identity, bias=bkv_sb[:, 0:1], scale=1.0,
            )
    with tc.tile_pool(name="pq1", bufs=2, space="PSUM") as pqp, \
         tc.tile_pool(name="tmp1", bufs=2) as tmp1:
        # RoPE on k (lives at partitions 64:128); roped k assembled at
        # parts 0:64 of kz0 (zeros above), duplicated to parts 64:128 of
        # kz1 (zeros below) — 128-partition stationaries stream columns at
        # full rate, a 64-partition stationary would run at half rate
        for ch in range(4):
            pr = pqp.tile([128, 512], F32, tag="rot")
            nc.tensor.matmul(
                pr[0:64, :], r2t_sb[64:128, 64:128],
                kvT_sb[64:128, ts(ch, 512)], start=True, stop=True,
            )
            t1 = tmp1.tile([128, 512], F32, tag="t1")
            t2 = tmp1.tile([128, 512], F32, tag="t2")
            nc.vector.tensor_mul(t1[0:64, :], kvT_sb[64:128, ts(ch, 512)], cos_sb[64:128, ts(ch, 512)])
            nc.vector.tensor_mul(t2[0:64, :], pr[0:64, :], sin_sb[0:64, ts(ch, 512)])
            nc.vector.tensor_add(kz0_sb[0:64, ts(ch, 512)], t1[0:64, :], t2[0:64, :])
        nc.vector.memset(kz0_sb[64:128, :], 0.0)
        nc.vector.memset(kz1_sb[0:64, :], 0.0)
        nc.sync.dma_start(out=kz1_sb[64:128, :], in_=kz0_sb[0:64, :])
        # q projection + bias + RoPE (in pair layout)
        def q_proj_rope(j):
            for ch in range(4):
                ps = pqp.tile([128, 512], F32, tag="proj", bufs=3, name="ps")
                for cc in range(NCC):
                    nc.tensor.matmul(
                        ps[:], wq_sb[:, cc, ts(j, 128)], xT_sb[:, cc, ts(ch, 512)],
                        start=(cc == 0), stop=(cc == NCC - 1),
                    )
                qp = tmp1.tile([128, 512], BF16, tag="qp", name="qp")
                nc.scalar.activation(
                    out=qp[:], in_=ps[:],
                    func=AF.Identity, bias=bq_sb[:, j:j + 1], scale=1.0,
                )
                pr = pqp.tile([128, 512], F32, tag="rot", name="pr")
                nc.tensor.matmul(pr[:], r2t_sb[:], qp[:], start=True, stop=True)
                t1 = tmp1.tile([128, 512], F32, tag="t1", name="t1")
                t2 = tmp1.tile([128, 512], F32, tag="t2", name="t2")
                nc.vector.tensor_mul(t1[:], qp[:], cos_sb[:, ts(ch, 512)])
                nc.vector.tensor_mul(t2[:], pr[:], sin_sb[:, ts(ch, 512)])
                nc.vector.tensor_add(qT_sb[:, j, ts(ch, 512)], t1[:], t2[:])

        q_proj_rope(0)
        # v -> [Tk, 64] layout with ones column (for the denominator).
        # Emitted after q pair 0's rope: vA is first needed LAG units into
        # the attention stream, so these DVE copies must not delay the
        # rope adds that gate the very first QK matmul.
        for c16 in range(16):
            pv = pqp.tile([128, 64], BF16, tag="vt")
            nc.tensor.transpose(pv[:], kvT_sb[0:64, ts(c16, 128)], id_sb[0:64, 0:64])
            nc.vector.tensor_copy(vA_sb[:, c16, 0:64], pv[:])
        nc.vector.memset(vA_sb[:, :, 64:65], 1.0)
        nc.vector.memset(vA_sb[:, :, 65:128], 0.0)
        q_proj_rope(1)

    # ---- phase 2: attention, software-pipelined ----
    with tc.tile_pool(name="pps", bufs=2, space="PSUM") as pps, \
         tc.tile_pool(name="ppy", bufs=2, space="PSUM") as ppy, \
         tc.tile_pool(name="ppb", bufs=1, space="PSUM") as ppb, \
         tc.tile_pool(name="ppo", bufs=1, space="PSUM") as ppo, \
         tc.tile_pool(name="expp", bufs=6) as expp, \
         tc.tile_pool(name="ost", bufs=3) as ost, \
         tc.tile_pool(name="nrm", bufs=2) as nrm:
        pending = deque()

        def pump(n=1):
            for _ in range(n):
                if pending:
                    pending.popleft()()

        def drain():
            while len(pending) > LAG:
                pump()

        reserve = []
        for qb in range(4):
            den_t = nrm.tile([1, 2, 2, 512], F32, tag="den")

            def mk_oproj_pair(tq, cf, qb=qb):
                # split out-projection: the j0 matmul only needs heads 0/1
                # (normalized by the first pair-recip), so it can fill the
                # PE while the second pair's reciprocal chain is in flight
                slot = {}

                def j0():
                    if qb == 3 and tq == 13:
                        # the score pool is idle in the tail — borrow a bank
                        # for two extra j0 prefills covering the final
                        # reciprocal round-trip
                        po = pps.tile([128, 1024], F32, tag="s", name="po")[:, 0:512]
                    else:
                        pool, tag = (ppy, "py") if qb == 3 and (tq + cf) % 2 else (ppo, "po")
                        po = pool.tile([128, 512], F32, tag=tag, name="po")
                    slot["po"] = po
                    nc.tensor.matmul(
                        po[:], yT_sb[:, 0, ts(tq, 128)], wo_sb[:, 0, ts(cf, 512)],
                        start=True, stop=False,
                    )

                def j1():
                    po = slot["po"]
                    nc.tensor.matmul(
                        po[:], yT_sb[:, 1, ts(tq, 128)], wo_sb[:, 1, ts(cf, 512)],
                        start=False, stop=True,
                    )
                    ob = ost.tile([128, 512], BF16, tag="ob", bufs=5)
                    if qb == 3 and (tq + cf) % 2:
                        nc.scalar.copy(out=ob[:], in_=po[:])
                    else:
                        nc.vector.tensor_copy(ob[:], po[:])
                    nc.sync.dma_start(out=d["out"].ap()[ts(tq, 128), ts(cf, 512)], in_=ob[:])

                return j0, j1

            tail_pairs = []
            for h in range(4):
                j, base = h // 2, (h % 2) * 64
                py = ppy.tile([128, 512], F32, tag="py")
                first_av = [True]

                def mk_av(py, kb, ecols, pycols, stop, first_av=first_av):
                    e_, (e0, e1) = ecols
                    p0, p1 = pycols

                    def go():
                        nc.tensor.matmul(
                            py[0:65, p0:p1], vA_sb[:, kb, 0:65], e_[:, e0:e1],
                            start=first_av[0], stop=stop,
                        )
                        first_av[0] = False
                    return go

                # full 128x512 blocks below the diagonal, two k-chunks per
                # unit sharing one 1024-wide score PSUM + exp
                kz = kz0_sb if h % 2 == 0 else kz1_sb
                for fk in range(2 * qb):
                    s_ = pps.tile([128, 1024], F32, tag="s")
                    e_ = expp.tile([128, 1024], BF16, tag="e")
                    for half in range(2):
                        kb = 2 * fk + half
                        nc.tensor.matmul(
                            s_[:, ts(half, 512)], kz[:, ts(kb, 128)],
                            qT_sb[:, j, ts(qb, 512)], start=True, stop=True,
                        )
                    nc.scalar.activation(out=e_[:], in_=s_[:], func=AF.Exp, scale=SCALE)
                    for half in range(2):
                        pending.append(mk_av(
                            py, 2 * fk + half, (e_, (512 * half, 512 * half + 512)),
                            (0, 512), stop=False))
                        drain()
                # diagonal band: k-chunk 4qb+r covers q in [(4qb+r)*128, (qb+1)*512);
                # its first 128 columns straddle the diagonal and get
                # tri-masked on the exp tile (GpSimd, SBUF side)
                for dpair in range(2):
                    s_ = pps.tile([128, 1024], F32, tag="s")
                    e_ = expp.tile([128, 1024], BF16, tag="e")
                    off = 0
                    offs = []
                    for idx in range(2):
                        r = 2 * dpair + idx
                        w = 512 - 128 * r
                        kb = 4 * qb + r
                        qoff = kb * 128
                        nc.tensor.matmul(
                            s_[:, off:off + w], kz[:, ts(kb, 128)],
                            qT_sb[:, j, qoff:qoff + w], start=True, stop=True,
                        )
                        offs.append(off)
                        off += w
                    nc.scalar.activation(out=e_[:, 0:off], in_=s_[:, 0:off], func=AF.Exp, scale=SCALE)
                    for idx in range(2):
                        o = offs[idx]
                        nc.vector.tensor_mul(e_[:, o:o + 128], e_[:, o:o + 128], tri_sb[:])
                    for idx in range(2):
                        r = 2 * dpair + idx
                        w = 512 - 128 * r
                        kb = 4 * qb + r
                        pending.append(mk_av(
                            py, kb, (e_, (offs[idx], offs[idx] + w)),
                            (128 * r, 512), stop=(r == 3)))
                        drain()

                # early evacuation: unnormalized y + den row leave PSUM
                # immediately so the py bank recycles without waiting on the
                # reciprocal chain
                def mk_evac(py=py, j=j, h=h, base=base, qb=qb, den_t=den_t):
                    def go():
                        nc.vector.tensor_copy(
                            yT_sb[base:base + 64, j, ts(qb, 512)], py[0:64, :])
                        nc.vector.tensor_copy(den_t[0:1, h // 2, h % 2, :], py[64:65, :])
                    return go

                pending.append(mk_evac())
                drain()

                if qb == 3 and h == 3:
                    # cover the final pair's reciprocal round-trip with four
                    # j0 out-proj matmuls (heads 0/1 already normed)
                    for tq in (12, 13):
                        for cf in range(2):
                            pair = mk_oproj_pair(tq, cf)
                            tail_pairs.append(pair)
                            pending.append(pair[0])

                if h % 2 == 1:
                    # per-head-pair normalization: DMA the [1,1024] den strip
                    # into a lane-parallel layout (any linearization —
                    # elementwise recip commutes with the permutation and the
                    # inverse DMA restores order), reciprocal, DMA back with
                    # the two heads on partitions 0/1, broadcast both heads'
                    # recips into one PSUM via a 2-row selector matmul, then
                    # normalize in place
                    def mk_norm_pair(c=h // 2, den_t=den_t, qb=qb):
                        slot = {}

                        def recip():
                            dtp = nrm.tile([128, 8], F32, tag="dtp")
                            nc.sync.dma_start(out=dtp[:], in_=den_t[0:1, c, :, :])
                            rtp = nrm.tile([128, 8], F32, tag="rtp")
                            nc.vector.reciprocal(rtp[:], dtp[:])
                            rdr = nrm.tile([2, 512], F32R, tag="rdr")
                            nc.sync.dma_start(out=rdr[0:2, :], in_=rtp[:].bitcast(F32R))
                            slot["rdr"] = rdr

                        def apply():
                            pb = ppb.tile([128, 512], F32, tag="pb", name="pb")
                            nc.tensor.matmul(
                                pb[:], bselp_sb[0:2, :], slot["rdr"][0:2, :],
                                start=True, stop=True,
                            )
                            for hh in (2 * c, 2 * c + 1):
                                jj, bb = hh // 2, (hh % 2) * 64
                                nc.vector.tensor_mul(
                                    yT_sb[bb:bb + 64, jj, ts(qb, 512)],
                                    yT_sb[bb:bb + 64, jj, ts(qb, 512)],
                                    pb[bb:bb + 64, :],
                                )
                        return recip, apply

                    rec_it, app_it = mk_norm_pair()
                    pending.append(rec_it)
                    pending.append(app_it)
                    drain()

            # out projection for this q-block (all 4 heads now normalized),
            # deferred into the next q-block's PE stream as filler
            if qb == 3:
                # drain the two pre-issued j0 accumulations, then the rest
                # j0/j1 adjacent (po double-buffered across ppo/ppy)
                for pair in tail_pairs:
                    pending.append(pair[1])
                for tq in range(14, 16):
                    for cf in range(2):
                        pair = mk_oproj_pair(tq, cf)
                        pending.append(pair[0])
                        pending.append(pair[1])
            else:
                for tq in range(4 * qb, 4 * qb + 4):
                    for cf in range(2):
                        pair = mk_oproj_pair(tq, cf)
                        pending.append(pair[0])
                        pending.append(pair[1])
            drain()
        while pending:
            pump()


def build_program():
    nc = bacc.Bacc("TRN2", target_bir_lowering=False, debug=False, num_devices=8)
    d = {}
    BF_IN = {"xT", "wq", "wkv", "wo", "r2t", "ident", "tri"}
    for name, shape in [
        ("xT", [C, T]), ("wq", [C, 256]), ("wkv", [C, 128]),
        ("bq", [128, 2]), ("bkv", [128, 1]), ("wo", [256, C]),
        ("cos2b", [128, T]), ("sin2b", [128, T]), ("r2t", [128, 128]),
        ("ident", [128, 128]), ("tri", [128, 128]), ("bselp", [2, 128]),
    ]:
        dt = BF16 if name in (BF_IN | {"cos2b", "sin2b"}) else (F32R if name == "bselp" else F32)
        d[name] = nc.dram_tensor(name, shape, dt, kind="ExternalInput")
    d["out"] = nc.dram_tensor("out", [T, C], BF16, kind="ExternalOutput")
    with tile.TileContext(nc) as tc, ExitStack() as ctx:
        _emit(nc, tc, ctx, d)
    nc.compile()
    return nc


def host_prep(inputs):
    """Slice/transpose the full inputs into the 8 per-core input maps."""
    import ml_dtypes
    bf = lambda a: np.ascontiguousarray(a).astype(ml_dtypes.bfloat16)
    f = lambda a: np.ascontiguousarray(np.asarray(a, dtype=np.float32))
    x, rc = f(inputs["x"]), f(inputs["rope_cache"])
    Wq, bq = f(inputs["Wq"]), f(inputs["bq"])
    Wk, bk = f(inputs["Wk"]), f(inputs["bk"])
    Wv, bv = f(inputs["Wv"]), f(inputs["bv"])
    Wo = f(inputs["Wo"])

    cos2 = np.tile(np.repeat(rc[:, 1::2].T, 2, axis=0), (2, 1))  # [128, T]
    sin2 = np.tile(np.repeat(rc[:, 0::2].T, 2, axis=0), (2, 1))
    R2 = np.zeros((128, 128), np.float32)
    for i in range(64):
        R2[2 * i, 2 * i + 1] = -1.0
        R2[2 * i + 1, 2 * i] = 1.0
    r2t = np.ascontiguousarray(R2.T)
    ident = np.eye(128, dtype=np.float32)
    kk, qq = np.arange(128)[:, None], np.arange(128)[None, :]
    tri = (kk <= qq).astype(np.float32)
    bselp = np.zeros((2, 128), np.float32)
    bselp[0, 0:64] = 1.0
    bselp[1, 64:128] = 1.0

    in_maps = []
    for core in range(8):
        b, g = core // 4, core % 4
        kv = g // 2
        in_maps.append({
            "xT": bf(x[b].T),
            "wq": bf(Wq[256 * g:256 * (g + 1), :].T),
            "wkv": bf(np.concatenate(
                [Wv[64 * kv:64 * (kv + 1)].T, Wk[64 * kv:64 * (kv + 1)].T], axis=1)),
            "bq": np.ascontiguousarray(bq[256 * g:256 * (g + 1)].reshape(2, 128).T),
            "bkv": np.concatenate(
                [bv[64 * kv:64 * (kv + 1)], bk[64 * kv:64 * (kv + 1)]]).reshape(128, 1),
            "wo": bf(Wo[:, 256 * g:256 * (g + 1)].T),
            "cos2b": bf(cos2), "sin2b": bf(sin2), "r2t": bf(r2t),
            "ident": bf(ident), "tri": bf(tri), "bselp": bselp,
        })
    return in_maps


_PROGRAM = None


def _get_program():
    global _PROGRAM
    if _PROGRAM is None:
        _PROGRAM = build_program()
    return _PROGRAM


def _gather(results, bo):
    full = np.empty((B, T, C), np.float32)
    for b in range(B):
        acc = results[4 * b]["out"].astype(np.float32).copy()
        for g in range(1, 4):
            acc += results[4 * b + g]["out"]
        full[b] = acc + bo
    return full


def kernel(**inputs):
    nc = _get_program()
    in_maps = host_prep(inputs)
    res = run_bass_kernel_spmd(nc, in_maps, list(range(8)))
    return _gather(res.results, np.asarray(inputs["bo"], np.float32))


def kernel_traced(**inputs):
    """Like kernel() but with NTFF tracing; returns (output, BassKernelResults)."""
    nc = _get_program()
    in_maps = host_prep(inputs)
    res = run_bass_kernel_spmd(nc, in_maps, list(range(8)), trace=True)
    return _gather(res.results, np.asarray(inputs["bo"], np.float32)), res


# revision 91
# speedup vs baseline: 1.0261x; 1.0117x over previous
"""Causal self-attention (GQA + RoPE) on 8 Trainium2 NeuronCores.

Sharding: core c = (b, g) with b = c // 4 (batch), g = c % 4 (group of 4
consecutive Q heads; KV head g // 2). Each core computes the attention
output for its 4 heads and a partial out-projection through the matching
256-column slice of Wo. Host sums the 4 partials per batch and adds bo.

Per-core kernel, tuned for PE-stream continuity (all matmul operands
bf16, fp32 PSUM accumulation):
  - kv projection loops the contraction chunk outermost so matmuls start
    as soon as the first x chunk lands from HBM (first chunk split into
    512-col DMA pieces so the very first matmul starts even earlier)
  - QK uses zero-padded 128-partition stationaries (kz0/kz1): the PE
    streams 1 column/cycle only with 128 active partitions; a 64-row
    stationary would halve throughput (measured 427ns vs 216ns per
    512-col matmul)
  - attention is emitted as software-pipelined units (2 QK matmuls into
    a 1024-wide score PSUM -> one 1024-wide exp -> 2 AV matmuls) with a
    pending-work deque keeping consumers ~LAG units behind producers so
    the PE never waits on the Activation engine
  - causal diag handled by multiplying exp tiles with a 0/1 triangle on
    DVE (bf16 SBUF operands hit the 2x DVE mode), off the PE/Act path
  - softmax denominator via a ones-column appended to V (stationary
    sliced to 65 output partitions); per-head-pair normalization: den
    rows DMAd to a lane-parallel layout (recip commutes with the
    round-trip permutation), one reciprocal, one 2-row selector matmul
    broadcasting both heads' recips into a single PSUM, in-place muls
  - unnormalized y and den rows leave PSUM immediately so py banks
    recycle without waiting on the reciprocal chain
  - per-qb out-projection deferred and pumped as PE filler during the
    next qb's attention; in the final q-block the idle py bank
    double-buffers the out-proj PSUM and the Act engine shares the
    evacuation load; output written bf16 (host gathers in f32)
"""

import sys

for _p in ("/opt/trn_rl_repo", "/opt/pypackages"):
    if _p not in sys.path:
        sys.path.append(_p)

from collections import deque
from contextlib import ExitStack

import numpy as np

import concourse.bacc as bacc
import concourse.mybir as mybir
import concourse.tile as tile
from concourse.bass import ts
from concourse.bass_utils import run_bass_kernel_spmd

B, T, C = 2, 2048, 1024
HQ, HKV, HD = 16, 2, 64
F32 = mybir.dt.float32
F32R = mybir.dt.float32r
BF16 = mybir.dt.bfloat16
AF = mybir.ActivationFunctionType
NCC = C // 128  # 8 chunks of the contraction dim
SCALE = 1.0 / 64.0  # the reference's double 1/sqrt(64) scaling
LAG = 5  # pending-work units the PE consumer stream trails producers by


def _emit(nc, tc, ctx, d):
    sing = ctx.enter_context(tc.tile_pool(name="sing", bufs=1))

    xT_sb = sing.tile([128, NCC, T], BF16)
    wq_sb = sing.tile([128, NCC, 256], BF16)
    wkv_sb = sing.tile([128, NCC, 128], BF16)
    wo_sb = sing.tile([128, 2, C], BF16)
    bq_sb = sing.tile([128, 2], F32)
    bkv_sb = sing.tile([128, 1], F32)
    cos_sb = sing.tile([128, T], BF16)
    sin_sb = sing.tile([128, T], BF16)
    r2t_sb = sing.tile([128, 128], BF16)
    id_sb = sing.tile([128, 128], BF16)
    tri_sb = sing.tile([128, 128], BF16)
    bselp_sb = sing.tile([2, 128], F32R)
    qT_sb = sing.tile([128, 2, T], BF16)   # pair j: head 2j at parts 0:64, 2j+1 at 64:128
    kvT_sb = sing.tile([128, T], BF16)     # v at parts 0:64, k (pre-rope) at 64:128
    kz0_sb = sing.tile([128, T], BF16)     # roped k at 0:64, zeros at 64:128
    kz1_sb = sing.tile([128, T], BF16)     # zeros at 0:64, roped k at 64:128
    vA_sb = sing.tile([128, 16, 128], BF16)  # v[k-chunk, :64] + ones col + zero pad
    yT_sb = sing.tile([128, 2, T], BF16)   # normalized attention out, pair layout

    # input DMAs: small weights/tables first so the first projection
    # matmuls start as early as possible; x streams in behind them;
    # wo (needed only by the out-projection) goes last
    xr = d["xT"].ap().rearrange("(cc p) t -> p cc t", p=128)
    nc.sync.dma_start(out=wkv_sb[:], in_=d["wkv"].ap().rearrange("(cc p) m -> p cc m", p=128))
    nc.sync.dma_start(out=bkv_sb[:], in_=d["bkv"].ap())
    for piece in range(4):
        nc.sync.dma_start(out=xT_sb[:, 0, ts(piece, 512)], in_=xr[:, 0, ts(piece, 512)])
    for cc in range(1, NCC):
        nc.sync.dma_start(out=xT_sb[:, cc, :], in_=xr[:, cc, :])
    nc.sync.dma_start(out=wq_sb[:], in_=d["wq"].ap().rearrange("(cc p) m -> p cc m", p=128))
    nc.sync.dma_start(out=bq_sb[:], in_=d["bq"].ap())
    nc.sync.dma_start(out=r2t_sb[:], in_=d["r2t"].ap())
    nc.sync.dma_start(out=cos_sb[:], in_=d["cos2b"].ap())
    nc.sync.dma_start(out=sin_sb[:], in_=d["sin2b"].ap())
    nc.sync.dma_start(out=id_sb[:], in_=d["ident"].ap())
    nc.sync.dma_start(out=tri_sb[:], in_=d["tri"].ap())
    nc.sync.dma_start(out=bselp_sb[:], in_=d["bselp"].ap())
    nc.sync.dma_start(out=wo_sb[:], in_=d["wo"].ap().rearrange("(j p) c -> p j c", p=128))

    # ---- phase 1: projections, RoPE, v transpose ----
    with tc.tile_pool(name="pkv", bufs=1, space="PSUM") as pkv:
        # kv projection, contraction-chunk outermost: matmuls start on the
        # first x chunk instead of waiting for all of x
        pskv = [pkv.tile([128, 512], F32, tag=f"kv{ch}", name=f"pskv{ch}") for ch in range(4)]
        for cc in range(NCC):
            for ch in range(4):
                nc.tensor.matmul(
                    pskv[ch][:], wkv_sb[:, cc, :], xT_sb[:, cc, ts(ch, 512)],
                    start=(cc == 0), stop=(cc == NCC - 1),
                )
        for ch in range(4):
            nc.scalar.activation(
                out=kvT_sb[:, ts(ch, 512)], in_=pskv[ch][:],
                func=AF.Identity, bias=bkv_sb[:, 0:1], scale=1.0,
            )
    with tc.tile_pool(name="pq1", bufs=2, space="PSUM") as pqp, \
         tc.tile_pool(name="tmp1", bufs=2) as tmp1:
        # RoPE on k (lives at partitions 64:128); roped k assembled at
        # parts 0:64 of kz0 (zeros above), duplicated to parts 64:128 of
        # kz1 (zeros below) — 128-partition stationaries stream columns at
        # full rate, a 64-partition stationary would run at half rate
        for ch in range(4):
            pr = pqp.tile([128, 512], F32, tag="rot")
            nc.tensor.matmul(
                pr[0:64, :], r2t_sb[64:128, 64:128],
                kvT_sb[64:128, ts(ch, 512)], start=True, stop=True,
            )
            t1 = tmp1.tile([128, 512], F32, tag="t1")
            t2 = tmp1.tile([128, 512], F32, tag="t2")
            nc.vector.tensor_mul(t1[0:64, :], kvT_sb[64:128, ts(ch, 512)], cos_sb[64:128, ts(ch, 512)])
            nc.vector.tensor_mul(t2[0:64, :], pr[0:64, :], sin_sb[0:64, ts(ch, 512)])
            nc.vector.tensor_add(kz0_sb[0:64, ts(ch, 512)], t1[0:64, :], t2[0:64, :])
        nc.vector.memset(kz0_sb[64:128, :], 0.0)
        nc.vector.memset(kz1_sb[0:64, :], 0.0)
        nc.sync.dma_start(out=kz1_sb[64:128, :], in_=kz0_sb[0:64, :])
        # q projection + bias + RoPE (in pair layout)
        def q_proj_rope(j):
            for ch in range(4):
                ps = pqp.tile([128, 512], F32, tag="proj", bufs=3, name="ps")
                for cc in range(NCC):
                    nc.tensor.matmul(
                        ps[:], wq_sb[:, cc, ts(j, 128)], xT_sb[:, cc, ts(ch, 512)],
                        start=(cc == 0), stop=(cc == NCC - 1),
                    )
                qp = tmp1.tile([128, 512], BF16, tag="qp", name="qp")
                nc.scalar.activation(
                    out=qp[:], in_=ps[:],
                    func=AF.Identity, bias=bq_sb[:, j:j + 1], scale=1.0,
                )
                pr = pqp.tile([128, 512], F32, tag="rot", name="pr")
                nc.tensor.matmul(pr[:], r2t_sb[:], qp[:], start=True, stop=True)
                t1 = tmp1.tile([128, 512], F32, tag="t1", name="t1")
                t2 = tmp1.tile([128, 512], F32, tag="t2", name="t2")
                nc.vector.tensor_mul(t1[:], qp[:], cos_sb[:, ts(ch, 512)])
                nc.vector.tensor_mul(t2[:], pr[:], sin_sb[:, ts(ch, 512)])
                nc.vector.tensor_add(qT_sb[:, j, ts(ch, 512)], t1[:], t2[:])

        q_proj_rope(0)
        # v -> [Tk, 64] layout with ones column (for the denominator).
        # Emitted after q pair 0's rope: vA is first needed LAG units into
        # the attention stream, so these DVE copies must not delay the
        # rope adds that gate the very first QK matmul.
        for c16 in range(16):
            pv = pqp.tile([128, 64], BF16, tag="vt")
            nc.tensor.transpose(pv[:], kvT_sb[0:64, ts(c16, 128)], id_sb[0:64, 0:64])
            nc.vector.tensor_copy(vA_sb[:, c16, 0:64], pv[:])
        nc.vector.memset(vA_sb[:, :, 64:65], 1.0)
        nc.vector.memset(vA_sb[:, :, 65:128], 0.0)
        q_proj_rope(1)

    # ---- phase 2: attention, software-pipelined ----
    with tc.tile_pool(name="pps", bufs=2, space="PSUM") as pps, \
         tc.tile_pool(name="ppy", bufs=2, space="PSUM") as ppy, \
         tc.tile_pool(name="ppb", bufs=1, space="PSUM") as ppb, \
         tc.tile_pool(name="ppo", bufs=1, space="PSUM") as ppo, \
         tc.tile_pool(name="expp", bufs=6) as expp, \
         tc.tile_pool(name="ost", bufs=3) as ost, \
         tc.tile_pool(name="nrm", bufs=2) as nrm:
        pending = deque()

        def pump(n=1):
            for _ in range(n):
                if pending:
                    pending.popleft()()

        def drain():
            while len(pending) > LAG:
                pump()

        reserve = []
        for qb in range(4):
            den_t = nrm.tile([1, 2, 2, 512], F32, tag="den")

            def mk_oproj_pair(tq, cf, qb=qb):
                # split out-projection: the j0 matmul only needs heads 0/1
                # (normalized by the first pair-recip), so it can fill the
                # PE while the second pair's reciprocal chain is in flight
                slot = {}

                def j0():
                    if qb == 3 and tq == 13:
                        # the score pool is idle in the tail — borrow a bank
                        # for two extra j0 prefills covering the final
                        # reciprocal round-trip
                        po = pps.tile([128, 1024], F32, tag="s", name="po")[:, 0:512]
                    else:
                        pool, tag = (ppy, "py") if qb == 3 and (tq + cf) % 2 else (ppo, "po")
                        po = pool.tile([128, 512], F32, tag=tag, name="po")
                    slot["po"] = po
                    nc.tensor.matmul(
                        po[:], yT_sb[:, 0, ts(tq, 128)], wo_sb[:, 0, ts(cf, 512)],
                        start=True, stop=False,
                    )

                def j1():
                    po = slot["po"]
                    nc.tensor.matmul(
                        po[:], yT_sb[:, 1, ts(tq, 128)], wo_sb[:, 1, ts(cf, 512)],
                        start=False, stop=True,
                    )
                    ob = ost.tile([128, 512], BF16, tag="ob", bufs=5)
                    if qb == 3 and (tq + cf) % 2:
                        nc.scalar.copy(out=ob[:], in_=po[:])
                    else:
                        nc.vector.tensor_copy(ob[:], po[:])
                    nc.sync.dma_start(out=d["out"].ap()[ts(tq, 128), ts(cf, 512)], in_=ob[:])

                return j0, j1

            tail_pairs = []
            for h in range(4):
                j, base = h // 2, (h % 2) * 64
                py = ppy.tile([128, 512], F32, tag="py")
                first_av = [True]

                def mk_av(py, kb, ecols, pycols, stop, first_av=first_av):
                    e_, (e0, e1) = ecols
                    p0, p1 = pycols

                    def go():
                        nc.tensor.matmul(
                            py[0:65, p0:p1], vA_sb[:, kb, 0:65], e_[:, e0:e1],
                            start=first_av[0], stop=stop,
                        )
                        first_av[0] = False
                    return go

                # full 128x512 blocks below the diagonal, two k-chunks per
                # unit sharing one 1024-wide score PSUM + exp
                kz = kz0_sb if h % 2 == 0 else kz1_sb
                for fk in range(2 * qb):
                    s_ = pps.tile([128, 1024], F32, tag="s")
                    e_ = expp.tile([128, 1024], BF16, tag="e")
                    for half in range(2):
                        kb = 2 * fk + half
                        nc.tensor.matmul(
                            s_[:, ts(half, 512)], kz[:, ts(kb, 128)],
                            qT_sb[:, j, ts(qb, 512)], start=True, stop=True,
                        )
                    nc.scalar.activation(out=e_[:], in_=s_[:], func=AF.Exp, scale=SCALE)
                    for half in range(2):
                        pending.append(mk_av(
                            py, 2 * fk + half, (e_, (512 * half, 512 * half + 512)),
                            (0, 512), stop=False))
                        drain()
                # diagonal band: k-chunk 4qb+r covers q in [(4qb+r)*128, (qb+1)*512);
                # its first 128 columns straddle the diagonal and get
                # tri-masked on the exp tile (GpSimd, SBUF side)
                for dpair in range(2):
                    s_ = pps.tile([128, 1024], F32, tag="s")
                    e_ = expp.tile([128, 1024], BF16, tag="e")
                    off = 0
                    offs = []
                    for idx in range(2):
                        r = 2 * dpair + idx
                        w = 512 - 128 * r
                        kb = 4 * qb + r
                        qoff = kb * 128
                        nc.tensor.matmul(
                            s_[:, off:off + w], kz[:, ts(kb, 128)],
                            qT_sb[:, j, qoff:qoff + w], start=True, stop=True,
                        )
                        offs.append(off)
                        off += w
                    nc.scalar.activation(out=e_[:, 0:off], in_=s_[:, 0:off], func=AF.Exp, scale=SCALE)
                    for idx in range(2):
                        o = offs[idx]
                        nc.vector.tensor_mul(e_[:, o:o + 128], e_[:, o:o + 128], tri_sb[:])
                    for idx in range(2):
                        r = 2 * dpair + idx
                        w = 512 - 128 * r
                        kb = 4 * qb + r
                        pending.append(mk_av(
                            py, kb, (e_, (offs[idx], offs[idx] + w)),
                            (128 * r, 512), stop=(r == 3)))
                        drain()

                # early evacuation: unnormalized y + den row leave PSUM
                # immediately so the py bank recycles without waiting on the
                # reciprocal chain
                def mk_evac(py=py, j=j, h=h, base=base, qb=qb, den_t=den_t):
                    def go():
                        nc.vector.tensor_copy(
                            yT_sb[base:base + 64, j, ts(qb, 512)], py[0:64, :])
                        nc.vector.tensor_copy(den_t[0:1, h // 2, h % 2, :], py[64:65, :])
                    return go

                pending.append(mk_evac())
                drain()

                if h % 2 == 1:
                    # per-head-pair normalization: DMA the [1,1024] den strip
                    # into a lane-parallel layout (any linearization —
                    # elementwise recip commutes with the permutation and the
                    # inverse DMA restores order), reciprocal, DMA back with
                    # the two heads on partitions 0/1, broadcast both heads'
                    # recips into one PSUM via a 2-row selector matmul, then
                    # normalize in place
                    def mk_norm_pair(c=h // 2, den_t=den_t, qb=qb):
                        slot = {}

                        def recip():
                            dtp = nrm.tile([128, 8], F32, tag="dtp")
                            nc.sync.dma_start(out=dtp[:], in_=den_t[0:1, c, :, :])
                            rtp = nrm.tile([128, 8], F32, tag="rtp")
                            nc.vector.reciprocal(rtp[:], dtp[:])
                            rdr = nrm.tile([2, 512], F32R, tag="rdr")
                            nc.sync.dma_start(out=rdr[0:2, :], in_=rtp[:].bitcast(F32R))
                            slot["rdr"] = rdr

                        def apply():
                            pb = ppb.tile([128, 512], F32, tag="pb", name="pb")
                            nc.tensor.matmul(
                                pb[:], bselp_sb[0:2, :], slot["rdr"][0:2, :],
                                start=True, stop=True,
                            )
                            for hh in (2 * c, 2 * c + 1):
                                jj, bb = hh // 2, (hh % 2) * 64
                                nc.vector.tensor_mul(
                                    yT_sb[bb:bb + 64, jj, ts(qb, 512)],
                                    yT_sb[bb:bb + 64, jj, ts(qb, 512)],
                                    pb[bb:bb + 64, :],
                                )
                        return recip, apply

                    rec_it, app_it = mk_norm_pair()
                    pending.append(rec_it)
                    if qb == 3 and h == 3:
                        # PE cover for the final reciprocal round-trip:
                        # complete out-proj pairs held back from qb2 (each
                        # frees its PSUM bank before the next allocates),
                        # then four j0 prefills (heads 0/1 already normed)
                        for pr_ in reserve:
                            pending.append(pr_[0])
                            pending.append(pr_[1])
                        for tq in (12, 13):
                            for cf in range(2):
                                pair = mk_oproj_pair(tq, cf)
                                tail_pairs.append(pair)
                                pending.append(pair[0])
                    pending.append(app_it)
                    drain()

            # out projection for this q-block (all 4 heads now normalized),
            # deferred into the next q-block's PE stream as filler
            if qb == 3:
                # drain the two pre-issued j0 accumulations, then the rest
                # j0/j1 adjacent (po double-buffered across ppo/ppy)
                for pair in tail_pairs:
                    pending.append(pair[1])
                for tq in range(14, 16):
                    for cf in range(2):
                        pair = mk_oproj_pair(tq, cf)
                        if qb == 2 and tq >= 10 and cf == 1:
                            reserve.append(pair)
                        else:
                            pending.append(pair[0])
                            pending.append(pair[1])
            else:
                for tq in range(4 * qb, 4 * qb + 4):
                    for cf in range(2):
                        pair = mk_oproj_pair(tq, cf)
                        if qb == 2 and tq >= 10 and cf == 1:
                            reserve.append(pair)
                        else:
                            pending.append(pair[0])
                            pending.append(pair[1])
            drain()
        while pending:
            pump()


def build_program():
    nc = bacc.Bacc("TRN2", target_bir_lowering=False, debug=False, num_devices=8)
    d = {}
    BF_IN = {"xT", "wq", "wkv", "wo", "r2t", "ident", "tri"}
    for name, shape in [
        ("xT", [C, T]), ("wq", [C, 256]), ("wkv", [C, 128]),
        ("bq", [128, 2]), ("bkv", [128, 1]), ("wo", [256, C]),
        ("cos2b", [128, T]), ("sin2b", [128, T]), ("r2t", [128, 128]),
        ("ident", [128, 128]), ("tri", [128, 128]), ("bselp", [2, 128]),
    ]:
        dt = BF16 if name in (BF_IN | {"cos2b", "sin2b"}) else (F32R if name == "bselp" else F32)
        d[name] = nc.dram_tensor(name, shape, dt, kind="ExternalInput")
    d["out"] = nc.dram_tensor("out", [T, C], BF16, kind="ExternalOutput")
    with tile.TileContext(nc) as tc, ExitStack() as ctx:
        _emit(nc, tc, ctx, d)
    nc.compile()
    return nc


def host_prep(inputs):
    """Slice/transpose the full inputs into the 8 per-core input maps."""
    import ml_dtypes
    bf = lambda a: np.ascontiguousarray(a).astype(ml_dtypes.bfloat16)
    f = lambda a: np.ascontiguousarray(np.asarray(a, dtype=np.float32))
    x, rc = f(inputs["x"]), f(inputs["rope_cache"])
    Wq, bq = f(inputs["Wq"]), f(inputs["bq"])
    Wk, bk = f(inputs["Wk"]), f(inputs["bk"])
    Wv, bv = f(inputs["Wv"]), f(inputs["bv"])
    Wo = f(inputs["Wo"])

    cos2 = np.tile(np.repeat(rc[:, 1::2].T, 2, axis=0), (2, 1))  # [128, T]
    sin2 = np.tile(np.repeat(rc[:, 0::2].T, 2, axis=0), (2, 1))
    R2 = np.zeros((128, 128), np.float32)
    for i in range(64):
        R2[2 * i, 2 * i + 1] = -1.0
        R2[2 * i + 1, 2 * i] = 1.0
    r2t = np.ascontiguousarray(R2.T)
    ident = np.eye(128, dtype=np.float32)
    kk, qq = np.arange(128)[:, None], np.arange(128)[None, :]
    tri = (kk <= qq).astype(np.float32)
    bselp = np.zeros((2, 128), np.float32)
    bselp[0, 0:64] = 1.0
    bselp[1, 64:128] = 1.0

    in_maps = []
    for core in range(8):
        b, g = core // 4, core % 4
        kv = g // 2
        in_maps.append({
            "xT": bf(x[b].T),
            "wq": bf(Wq[256 * g:256 * (g + 1), :].T),
            "wkv": bf(np.concatenate(
                [Wv[64 * kv:64 * (kv + 1)].T, Wk[64 * kv:64 * (kv + 1)].T], axis=1)),
            "bq": np.ascontiguousarray(bq[256 * g:256 * (g + 1)].reshape(2, 128).T),
            "bkv": np.concatenate(
                [bv[64 * kv:64 * (kv + 1)], bk[64 * kv:64 * (kv + 1)]]).reshape(128, 1),
            "wo": bf(Wo[:, 256 * g:256 * (g + 1)].T),
            "cos2b": bf(cos2), "sin2b": bf(sin2), "r2t": bf(r2t),
            "ident": bf(ident), "tri": bf(tri), "bselp": bselp,
        })
    return in_maps


_PROGRAM = None


def _get_program():
    global _PROGRAM
    if _PROGRAM is None:
        _PROGRAM = build_program()
    return _PROGRAM


def _gather(results, bo):
    full = np.empty((B, T, C), np.float32)
    for b in range(B):
        acc = results[4 * b]["out"].astype(np.float32).copy()
        for g in range(1, 4):
            acc += results[4 * b + g]["out"]
        full[b] = acc + bo
    return full


def kernel(**inputs):
    nc = _get_program()
    in_maps = host_prep(inputs)
    res = run_bass_kernel_spmd(nc, in_maps, list(range(8)))
    return _gather(res.results, np.asarray(inputs["bo"], np.float32))


def kernel_traced(**inputs):
    """Like kernel() but with NTFF tracing; returns (output, BassKernelResults)."""
    nc = _get_program()
    in_maps = host_prep(inputs)
    res = run_bass_kernel_spmd(nc, in_maps, list(range(8)), trace=True)
    return _gather(res.results, np.asarray(inputs["bo"], np.float32)), res
